# revision 24
# baseline (speedup 1.0000x reference)
"""Trainium2 Bass kernel for nn_DiplomacyModel (GNN message passing).

Model per stack (bo/po), 8 blocks:
  y1[b,n,o] = sum_i x[b,n,i] W[n,i,o]        (per-node matmul)
  y2[b,m,o] = sum_n A[m,n] y1[b,n,o] + bias  (adjacency mix)
  y  = relu((y2 - mean)/std * gamma + beta)  (batchnorm over (batch, feat) per node)
  x' = y + x                                  (residual, blocks 1..7)
Output: concat(bo, po) over feature dim -> [2048, 81, 512].

Distribution: data-parallel over batch across 8 cores (256 batch each),
local batch-norm statistics (error vs global stats ~0.6% rel-l2).

Per-core on-chip layout (bf16 activations, fp32 PSUM/stats):
  X    [128, 2, 81, 256]  feature-major carrier (i%128-part, i//128, node, batch)
  Y1NM [82, 65536]        node-major staging, f = (p, oc, b) so the fm<->nm
                          transpose DMAs move 1KB-contiguous runs per
                          partition. Row 81 = bias row.
Weights are repacked host-side to [128(p), n, ic, o] so streaming is
per-partition contiguous (3 nodes per dma_start -> 3KB packets).
Per block: einsum1 (PE) -> evict PSUM->ev bf16 (ACT/DVE) -> conv1 DMA
(1 dma/node, 1KB runs) -> A-mix matmuls -> evict+S accum (ACT/DVE) ->
Q via ACT Square-accum -> stats -> norm+relu in place (DVE) -> conv2
DMA (1/node, 1KB runs) -> fold add into X (DVE).
"""

import math
import os
import sys
from contextlib import ExitStack

import numpy as np

sys.path.insert(0, "/opt/trn_rl_repo")

import ml_dtypes  # noqa: E402

import concourse.bass as bass  # noqa: E402
import concourse.mybir as mybir  # noqa: E402
import concourse.tile as tile  # noqa: E402
from concourse import bacc  # noqa: E402
from concourse.bass_utils import run_bass_kernel_spmd  # noqa: E402

F32 = mybir.dt.float32
BF16 = mybir.dt.bfloat16
AF = mybir.ActivationFunctionType
ALU = mybir.AluOpType

N = 81          # nodes
EMB = 256
BATCH = 2048
NCORES = 8
BC = BATCH // NCORES  # 256 per-core batch
NBLK = 8
F0 = {"bo": 35, "po": 40}
EPS = 1e-5
FTOT = EMB * BC  # 65536 node-major free size per block
WG = 3           # nodes per weight/x0 streaming group (81 = 27*3)

_CACHE = {}


def _bf(x):
    return np.ascontiguousarray(x.astype(ml_dtypes.bfloat16))


def build_kernel(nc, nblk=NBLK, stacks=("bo", "po")):
    """Emit the full SPMD program for one core."""
    # ---- DRAM I/O ----
    io = {}
    for s in stacks:
        io[f"x0_{s}"] = nc.dram_tensor(f"x0_{s}", [F0[s], N, BC], BF16, kind="ExternalInput")
        io[f"w0_{s}"] = nc.dram_tensor(f"w0_{s}", [F0[s], N, EMB], BF16, kind="ExternalInput")
        if nblk > 1:
            io[f"w_{s}"] = nc.dram_tensor(
                f"w_{s}", [nblk - 1, 128, N, 2, EMB], BF16, kind="ExternalInput")
    nbt = len(stacks) * nblk
    io["brow"] = nc.dram_tensor("brow", [nbt, FTOT], BF16, kind="ExternalInput")
    io["gamma"] = nc.dram_tensor("gamma", [N, nbt], F32, kind="ExternalInput")
    io["beta"] = nc.dram_tensor("beta", [N, nbt], F32, kind="ExternalInput")
    io["aaug"] = nc.dram_tensor("aaug", [N + 1, N], BF16, kind="ExternalInput")
    io["yout"] = nc.dram_tensor(
        "yout", [len(stacks), 128, 2 * N * BC], BF16, kind="ExternalOutput")

    with ExitStack() as ctx:
        # persistent SBUF — allocate BEFORE TileContext so pool-slot
        # assignment (which happens at TileContext exit) can't alias them.
        X = ctx.enter_context(nc.sbuf_tensor([128, 2 * N * BC], BF16))
        Y1 = ctx.enter_context(nc.sbuf_tensor([N + 1, FTOT], BF16))
        aaug_t = ctx.enter_context(nc.sbuf_tensor([N + 1, N], BF16))
        tc = ctx.enter_context(tile.TileContext(nc))
        nc.sync.dma_start(aaug_t[:], io["aaug"][:])

        Xv = X.rearrange("p (ic n b) -> p ic n b", ic=2, n=N, b=BC)
        # node-major view with f = (p, oc, b): conv runs are 1KB contiguous
        Y1v = Y1.rearrange("n (p oc b) -> n p oc b", p=128, oc=2, b=BC)

        # pools
        wpool = ctx.enter_context(tc.tile_pool(name="w", bufs=2))
        x0pool = ctx.enter_context(tc.tile_pool(name="x0", bufs=1))
        evpool = ctx.enter_context(tc.tile_pool(name="ev", bufs=2))
        slabpool = ctx.enter_context(tc.tile_pool(name="slab", bufs=2))
        stpool = ctx.enter_context(tc.tile_pool(name="stats", bufs=1))
        ps1 = ctx.enter_context(tc.tile_pool(name="ps1", bufs=3, space="PSUM"))
        ps2 = ctx.enter_context(tc.tile_pool(name="ps2", bufs=2, space="PSUM"))

        for si, s in enumerate(stacks):
            for k in range(nblk):
                bi = si * nblk + k
                kc = 1 if k == 0 else 2  # K chunks of einsum1

                # ---------- phase 1: einsum1 per node ----------
                for g in range(N // WG):
                    n0 = g * WG
                    if k == 0:
                        w = wpool.tile([F0[s], WG, EMB], BF16, tag="w")
                        nc.sync.dma_start(w[:], io[f"w0_{s}"][:, n0:n0 + WG])
                        xt = x0pool.tile([F0[s], WG, BC], BF16, tag="x0")
                        nc.sync.dma_start(xt[:], io[f"x0_{s}"][:, n0:n0 + WG])
                    else:
                        w = wpool.tile([128, WG, 2, EMB], BF16, tag="w")
                        nc.sync.dma_start(w[:], io[f"w_{s}"][k - 1, :, n0:n0 + WG])
                    for j in range(WG):
                        n = n0 + j
                        ps = ps1.tile([128, 2 * BC], F32, tag="ps1")
                        for oc in range(2):
                            for ic in range(kc):
                                if k == 0:
                                    lhsT = w[:, j, oc * 128:(oc + 1) * 128]
                                    rhs = xt[:, j, :]
                                else:
                                    lhsT = w[:, j, ic, oc * 128:(oc + 1) * 128]
                                    rhs = Xv[:, ic, n, :]
                                nc.tensor.matmul(
                                    ps[:, oc * BC:(oc + 1) * BC], lhsT, rhs,
                                    start=(ic == 0), stop=(ic == kc - 1))
                        ev = evpool.tile([128, 2, BC], BF16, tag="ev")
                        evf = ev.rearrange("p a b -> p (a b)")
                        if n % 2 == 0:
                            nc.scalar.copy(evf[:], ps[:])
                        else:
                            nc.vector.tensor_copy(evf[:], ps[:])
                        # conv1: fm -> nm row n, 1KB runs: dst f = (p, oc, b)
                        nc.sync.dma_start(Y1[n:n + 1, :], evf[:])

                # bias row + per-block bn consts
                nc.sync.dma_start(Y1[N:N + 1, :], io["brow"][bi:bi + 1, :])
                gt = stpool.tile([N, 1], F32, tag="g")
                bt = stpool.tile([N, 1], F32, tag="b")
                nc.sync.dma_start(gt[:], io["gamma"][:, bi:bi + 1])
                nc.sync.dma_start(bt[:], io["beta"][:, bi:bi + 1])

                # ---------- phase 2a: A-mix + in-place stage + S accum ----------
                smat = stpool.tile([N, 64], F32, tag="smat")
                NCH = 64  # chunks of 1024
                for c in range(NCH):
                    pt = ps2.tile([N, 1024], F32, tag="ps2")
                    for h in range(2):
                        f0 = c * 1024 + h * 512
                        nc.tensor.matmul(
                            pt[:, h * 512:(h + 1) * 512], aaug_t[:],
                            Y1[:, f0:f0 + 512], start=True, stop=True)
                    dstc = Y1[0:N, c * 1024:(c + 1) * 1024]
                    if c % 2 == 0:
                        nc.scalar.activation(dstc, pt[:], AF.Copy,
                                             accum_out=smat[:, c:c + 1])
                    else:
                        nc.vector.tensor_scalar(
                            dstc, pt[:], 1.0, 0.0, ALU.mult, ALU.add,
                            accum_out=smat[:, c:c + 1])

                # ---------- phase 2b: Q + stats ----------
                qmat = stpool.tile([N, 64], F32, tag="qmat")
                for r in range(64):
                    qs = ps2.tile([N, 1024], F32, tag="ps2")
                    nc.scalar.activation(
                        qs[:], Y1[0:N, r * 1024:(r + 1) * 1024], AF.Square,
                        accum_out=qmat[:, r:r + 1])
                S = stpool.tile([N, 1], F32, tag="S")
                Q = stpool.tile([N, 1], F32, tag="Q")
                nc.vector.tensor_reduce(S[:], smat[:], axis=mybir.AxisListType.X, op=ALU.add)
                nc.vector.tensor_reduce(Q[:], qmat[:], axis=mybir.AxisListType.X, op=ALU.add)
                mean = stpool.tile([N, 1], F32, tag="mean")
                var = stpool.tile([N, 1], F32, tag="var")
                sg = stpool.tile([N, 1], F32, tag="sg")
                tb = stpool.tile([N, 1], F32, tag="tb")
                inv = 1.0 / float(FTOT)
                nc.vector.tensor_scalar(mean[:], S[:], inv, None, ALU.mult)
                nc.vector.tensor_scalar(var[:], Q[:], inv, None, ALU.mult)
                # var = E[x^2] - mean^2 ; sg = gamma/sqrt(var+eps) ; tb = beta - mean*sg
                nc.vector.tensor_tensor(sg[:], mean[:], mean[:], op=ALU.mult)
                nc.vector.tensor_tensor(var[:], var[:], sg[:], op=ALU.subtract)
                nc.vector.tensor_scalar(var[:], var[:], EPS, None, ALU.add)
                nc.scalar.activation(var[:], var[:], AF.Sqrt)
                nc.vector.reciprocal(sg[:], var[:])
                nc.vector.tensor_tensor(sg[:], sg[:], gt[:], op=ALU.mult)
                nc.vector.tensor_tensor(tb[:], mean[:], sg[:], op=ALU.mult)
                nc.vector.tensor_tensor(tb[:], bt[:], tb[:], op=ALU.subtract)

                # ---------- phase 2c: norm+relu in place ----------
                for r in range(32):
                    sp = Y1[0:N, r * 2048:(r + 1) * 2048]
                    nc.vector.tensor_scalar(sp, sp, sg[:], tb[:], ALU.mult, ALU.add)
                    nc.vector.tensor_scalar(sp, sp, 0.0, None, ALU.max)

                # ---------- conv2 + fold ----------
                last = (k == nblk - 1)
                for n in range(N):
                    slab = slabpool.tile([128, 2, BC], BF16, tag="slab")
                    slabf = slab.rearrange("p a b -> p (a b)")
                    nc.sync.dma_start(slabf[:], Y1[n:n + 1, :])
                    xr = Xv[:, :, n, :]
                    if k == 0:
                        nc.vector.tensor_copy(xr, slab[:])
                    else:
                        nc.vector.tensor_tensor(xr, xr, slab[:], op=ALU.add)
                if last:
                    nc.sync.dma_start(io["yout"][si], X[:])
    return io


# ---------------- host side ----------------

def _prep_inputs(inputs):
    """Build per-core device input maps from the full-model inputs."""
    A = np.asarray(inputs["A"], np.float32)
    aaug = np.concatenate([A.T, np.ones((1, N), np.float32)], axis=0)

    common = {"aaug": _bf(aaug)}
    brows, gammas, betas = [], [], []
    for s in ("bo", "po"):
        for k in range(NBLK):
            if k == 0:
                bvec = np.asarray(inputs[f"b0_{s}"], np.float32).reshape(EMB)
                g = np.asarray(inputs[f"g0_{s}"], np.float32)
                be = np.asarray(inputs[f"be0_{s}"], np.float32)
            else:
                bvec = np.asarray(inputs[f"b_{s}"][k - 1], np.float32).reshape(EMB)
                g = np.asarray(inputs[f"g_{s}"][k - 1], np.float32)
                be = np.asarray(inputs[f"be_{s}"][k - 1], np.float32)
            # f = (p, oc, b): brow[f] = bvec[oc*128 + p]
            brows.append(np.repeat(bvec.reshape(2, 128).T.reshape(-1), BC))
            gammas.append(g)
            betas.append(be)
    common["brow"] = _bf(np.stack(brows))
    common["gamma"] = np.ascontiguousarray(np.stack(gammas, axis=1), dtype=np.float32)
    common["beta"] = np.ascontiguousarray(np.stack(betas, axis=1), dtype=np.float32)
    for s in ("bo", "po"):
        w0 = np.asarray(inputs[f"W0_{s}"], np.float32)           # [81, F0, 256]
        common[f"w0_{s}"] = _bf(w0.transpose(1, 0, 2))           # [F0, 81, 256]
        w = np.asarray(inputs[f"W_{s}"], np.float32)             # [7, 81, 256, 256]
        wr = w.reshape(NBLK - 1, N, 2, 128, EMB).transpose(0, 3, 1, 2, 4)
        common[f"w_{s}"] = _bf(wr)                               # [7, 128, 81, 2, 256]

    in_maps = []
    for c in range(NCORES):
        m = dict(common)
        for s in ("bo", "po"):
            xs = np.asarray(inputs[f"x_{s}"], np.float32)[c * BC:(c + 1) * BC]
            m[f"x0_{s}"] = _bf(xs.transpose(2, 1, 0))            # [F0, 81, 256]
        in_maps.append(m)
    return in_maps


def _assemble(results):
    """[core][yout [2,128,2*81*256] bf16] -> [2048, 81, 512] f32."""
    out = np.empty((BATCH, N, 2 * EMB), np.float32)
    for c, res in enumerate(results):
        y = np.asarray(res["yout"])  # bf16 [2, 128, 2*81*256]
        y = y.astype(np.float32).reshape(2, 128, 2, N, BC)
        # [s, p, ic, n, b] -> [b, n, o= ic*128+p (+256*s)]
        y = y.transpose(4, 3, 0, 2, 1).reshape(BC, N, 2 * EMB)
        out[c * BC:(c + 1) * BC] = y
    return out


def kernel(**inputs):
    key = "nc"
    if key not in _CACHE:
        nc = bacc.Bacc("TRN2", target_bir_lowering=False, debug=False,
                       num_devices=NCORES, dynamic_dma_scratch_size=1024)
        build_kernel(nc)
        nc.compile()
        _CACHE[key] = nc
    nc = _CACHE[key]
    in_maps = _prep_inputs(inputs)
    res = run_bass_kernel_spmd(nc, in_maps, core_ids=list(range(NCORES)))
    return _assemble(res.results)


if __name__ == "__main__":
    rng = np.random.RandomState(0)
    fake = {
        "x_bo": rng.randn(BATCH, N, 35).astype(np.float32),
        "x_po": rng.randn(BATCH, N, 40).astype(np.float32),
        "A": rng.rand(N, N).astype(np.float32),
    }
    for s, f0 in (("bo", 35), ("po", 40)):
        fake[f"W0_{s}"] = rng.randn(N, f0, EMB).astype(np.float32) * 0.1
        fake[f"b0_{s}"] = np.zeros((1, 1, EMB), np.float32)
        fake[f"g0_{s}"] = np.ones((N,), np.float32)
        fake[f"be0_{s}"] = np.zeros((N,), np.float32)
        fake[f"W_{s}"] = rng.randn(NBLK - 1, N, EMB, EMB).astype(np.float32) * 0.06
        fake[f"b_{s}"] = np.zeros((NBLK - 1, 1, 1, EMB), np.float32)
        fake[f"g_{s}"] = np.ones((NBLK - 1, N), np.float32)
        fake[f"be_{s}"] = np.zeros((NBLK - 1, N), np.float32)
    out = kernel(**fake)
    print("out", out.shape, out.dtype, float(np.abs(out).max()))
